# revision 1
# baseline (speedup 1.0000x reference)
"""Multi-head self-attention Bass/Tile kernel for Trainium2, SPMD over 8 cores.

Problem: B=2, T=4096, D=768, H=12, HD=64 dense MHSA (full TxT scores,
key-padding mask, softmax, out-proj with bias).

Sharding: core c handles batch b=c//4 and query slice q0=(c%4)*1024 for all
12 heads.  No collectives: each core computes a disjoint [768, 1024] slice of
the (transposed) output; the host gathers.

Key optimizations vs the naive layout:
  * Host-side key compaction: ~half the keys are masked out (-inf) in the
    reference; we gather the unmasked key columns of x^T on the host and the
    device kernel only sees NK ~ 2176 keys instead of 4096.  Pad keys have
    zero x columns (so zero K and V rows) and a 0/1 "mcol" vector excludes
    them from the softmax denominator, so no exp bias / mask handling is
    needed on device at all.
  * S = K_h^T.T @ Q_h^T per (head, key-tile) -> PSUM [128k, 1024q], exp on
    ACT -> P (bf16, SBUF).
  * AV is "swapped": O[q, f] accumulates via lhsT=P-chunk [128k, 128q],
    rhs=[V_h | mcol] streaming only 65 columns per (head, key-tile, q-tile)
    instead of streaming 512 query columns.  The denominator comes out as
    column 64 of O for free.
  * Normalize = per-partition scalar multiply on DVE (O is query-major), then
    a PE transpose brings O back to feature-major for the out-projection.
  * Q/K^T/V projections and the out-projection run with full 128-row output
    tiles (head pairs packed), everything SBUF-resident (no DRAM staging).
"""

import functools
import math

import numpy as np

import concourse.bass as bass
import concourse.mybir as mybir
import concourse.tile as tile
from concourse import bacc
from concourse.bass2jax import (
    _bass_exec_p,
    install_neuronx_cc_hook,
    partition_id_tensor,
)

F32 = mybir.dt.float32
BF16 = mybir.dt.bfloat16
MMDT = BF16
AF = mybir.ActivationFunctionType

B, T, D = 2, 4096, 768
H, HD = 12, 64
N_CORES = 8
CORES_PER_B = 4
QS = T // CORES_PER_B           # 1024 query tokens per core
DT = D // 128                   # 6 feature tiles
QC = QS // 512                  # 2 query chunks of 512
QT8 = QS // 128                 # 8 query tiles of 128
DEFAULT_NKT = 17                # key tiles after compaction (seed-0 inputs)

# kt ranges processed per segment of phase B; KV tiles for segment i+1 are
# produced while segment i's heads run (ACT-bound), PSUM only ever holds one
# head's O accumulator.
def _segments(nkt: int, nseg):
    if isinstance(nseg, tuple):          # explicit sizes
        segs, k = [], 0
        for s in nseg:
            if k >= nkt:
                break
            segs.append((k, min(k + s, nkt)))
            k += s
        if k < nkt:
            segs.append((k, nkt))
        return segs
    per = (nkt + nseg - 1) // nseg
    segs = []
    k = 0
    while k < nkt:
        segs.append((k, min(k + per, nkt)))
        k += per
    return segs


def build_program(reps: int = 1, nkt: int = DEFAULT_NKT,
                  nseg: int | None = None):
    nseg = N_SEG if nseg is None else nseg
    nc = bacc.Bacc("TRN2", target_bir_lowering=False, debug=False,
                   num_devices=N_CORES)
    nk = nkt * 128

    xTq = nc.dram_tensor("xTq", [D, QS], MMDT, kind="ExternalInput").ap()
    xTk = nc.dram_tensor("xTk", [D, nk], MMDT, kind="ExternalInput").ap()
    wqT = nc.dram_tensor("wqT", [D, D], MMDT, kind="ExternalInput").ap()
    wkT = nc.dram_tensor("wkT", [D, D], MMDT, kind="ExternalInput").ap()
    wvT = nc.dram_tensor("wvT", [D, D], MMDT, kind="ExternalInput").ap()
    wpT = nc.dram_tensor("wpT", [D, D], MMDT, kind="ExternalInput").ap()
    bp = nc.dram_tensor("bp", [128, DT], F32, kind="ExternalInput").ap()
    mcol = nc.dram_tensor("mcol", [128, nkt], MMDT, kind="ExternalInput").ap()
    ident = nc.dram_tensor("ident", [128, 128], MMDT,
                           kind="ExternalInput").ap()
    outT = nc.dram_tensor("outT", [D, QS], MMDT, kind="ExternalOutput").ap()

    with tile.TileContext(nc) as tc, nc.allow_low_precision(
            reason="bf16 matmul pipeline"):
        def emit_once():
            _body(nc, tc, nkt, nseg, xTq, xTk, wqT, wkT, wvT, wpT, bp, mcol,
                  ident, outT)
        if reps == 1:
            emit_once()
        elif reps < 0:
            for _ in range(-reps):
                emit_once()
        else:
            with tc.For_i(0, reps, 1):
                emit_once()
    nc.compile()
    return nc


def _body(nc, tc, nkt, nseg, xTq, xTk, wqT, wkT, wvT, wpT, bp, mcol, ident,
          outT):
    from contextlib import ExitStack

    nk = nkt * 128
    segs = _segments(nkt, nseg)

    with ExitStack() as root:
        # ---------------- SBUF pools (all root-scoped; it fits) -----------
        const = root.enter_context(tc.tile_pool(name="const", bufs=1))
        w_pool = root.enter_context(tc.tile_pool(name="w", bufs=1))
        x_pool = root.enter_context(tc.tile_pool(name="x", bufs=1))
        qkv_pool = root.enter_context(tc.tile_pool(name="qkv", bufs=1))
        p_pool = root.enter_context(tc.tile_pool(name="p", bufs=15))
        nrm_pool = root.enter_context(tc.tile_pool(name="nrm", bufs=6))
        ot_pool = root.enter_context(tc.tile_pool(name="ot", bufs=1))
        spill_pool = root.enter_context(tc.tile_pool(name="spl", bufs=1))
        ost_pool = root.enter_context(tc.tile_pool(name="ost", bufs=3))

        # DMA transfers all serialize through one modeled DMA pipe
        # (~360GB/s), and each issuing engine pays per-DMA setup on its own
        # sequencer/engine.  So: split out exactly the slices the first
        # S(h0, kt0) chain needs, issue them first across SP/ACT/DVE, and
        # push the bulk through Pool afterwards in need-order.
        wq_sb, wk_sb, wv_sb, wp_sb = [], [], [], []
        for lst, nm in ((wq_sb, "wq"), (wk_sb, "wk"), (wv_sb, "wv"),
                        (wp_sb, "wp")):
            for d in range(DT):
                w = w_pool.tile([128, D], MMDT, tag=f"{nm}{d}", name=f"{nm}{d}")
                lst.append(w)
        xq_sb = [x_pool.tile([128, QS], MMDT, tag=f"xq{d}", name=f"xq{d}")
                 for d in range(DT)]
        xk_sb = [x_pool.tile([128, nk], MMDT, tag=f"xk{d}", name=f"xk{d}")
                 for d in range(DT)]
        bp_sb = const.tile([128, DT], F32, tag="bp")
        mcol_sb = const.tile([128, nkt], MMDT, tag="mcol")
        id_sb = const.tile([128, 128], MMDT, tag="ident")

        # per-head O landing zone in SBUF: segments accumulate into it; the
        # final normalize reads it, so the PSUM O slot frees after one copy
        spl = [spill_pool.tile([128, QT8 * (HD + 1)], F32, tag=f"spl{h}",
                               name=f"spl{h}")
               for h in range(H)]

        for d in range(DT):   # critical: Q pair-0 c0 operands
            nc.sync.dma_start(xq_sb[d][:, 0:512],
                              xTq[d * 128:(d + 1) * 128, 0:512])
            nc.sync.dma_start(wq_sb[d][:, 0:128],
                              wqT[d * 128:(d + 1) * 128, 0:128])
        for d in range(DT):   # critical: K-tile-0 operands + mcol
            nc.scalar.dma_start(wk_sb[d][:, 0:128],
                                wkT[d * 128:(d + 1) * 128, 0:128])
            nc.scalar.dma_start(xk_sb[d][:, 0:512],
                                xTk[d * 128:(d + 1) * 128, 0:512])
        nc.scalar.dma_start(mcol_sb[:], mcol[:])
        for d in range(DT):   # near-critical: rest of Q inputs
            nc.sync.dma_start(xq_sb[d][:, 512:QS],
                              xTq[d * 128:(d + 1) * 128, 512:QS])
        dma = nc.gpsimd.dma_start
        for d in range(DT):   # bulk: K inputs first (gate chunk production)
            dma(xk_sb[d][:, 512:nk], xTk[d * 128:(d + 1) * 128, 512:nk])
            dma(wk_sb[d][:, 128:D], wkT[d * 128:(d + 1) * 128, 128:D])
        for d in range(DT):
            dma(wq_sb[d][:, 128:D], wqT[d * 128:(d + 1) * 128, 128:D])
        for d in range(DT):
            dma(wv_sb[d][:], wvT[d * 128:(d + 1) * 128, :])
        dma(id_sb[:], ident[:])
        dma(bp_sb[:], bp[:])
        for d in range(DT):
            dma(wp_sb[d][:], wpT[d * 128:(d + 1) * 128, :])

        # long-lived activations
        qT6 = [qkv_pool.tile([128, QS], MMDT, tag=f"qT{p}", name=f"qT{p}")
               for p in range(DT)]
        kT6 = [qkv_pool.tile([128, nk], MMDT, tag=f"kT{p}", name=f"kT{p}")
               for p in range(DT)]
        vp_sb = qkv_pool.tile([128, nkt * D], MMDT, tag="vp", name="vp")
        vp3 = vp_sb[:].rearrange("p (kt e) -> p kt e", e=D)
        otT6 = [ot_pool.tile([128, QS], MMDT, tag=f"otT{p}", name=f"otT{p}")
                for p in range(DT)]

        # ---------------- PSUM pools --------------------------------------
        # qps scope closes before the S/O pools open: 2 + (2+4+2) <= 8 banks.
        kv_ps = root.enter_context(
            tc.tile_pool(name="kvps", bufs=2, space="PSUM"))

        def emit_q_pair(p, cp=None):
            cp = cp or nc.vector.tensor_copy
            for c in range(QC):
                ps = kv_ps.tile([128, 512], F32, tag="kv", name="qps")
                for d in range(DT):
                    nc.tensor.matmul(
                        ps[:], wq_sb[d][:, p * 128:(p + 1) * 128],
                        xq_sb[d][:, c * 512:(c + 1) * 512],
                        start=(d == 0), stop=(d == DT - 1))
                cp(qT6[p][:, c * 512:(c + 1) * 512], ps[:])

        # K^T rows (e-tiles) for key span [k0, k1), width <= 512
        def emit_k(k0, k1, es):
            w = k1 - k0
            for e in es:
                kps = kv_ps.tile([128, 512], F32, tag="kv", name="kps")
                for d in range(DT):
                    nc.tensor.matmul(
                        kps[:, 0:w], wk_sb[d][:, e * 128:(e + 1) * 128],
                        xk_sb[d][:, k0:k1],
                        start=(d == 0), stop=(d == DT - 1))
                nc.vector.tensor_copy(kT6[e][:, k0:k1], kps[:, 0:w])

        # V rows for key span [k0, k1) (multiples of 128)
        def emit_v(k0, k1, cp=None):
            cp = cp or nc.vector.tensor_copy
            for tt in range(k0 // 128, k1 // 128):
                tsl = slice(tt * 128, (tt + 1) * 128)
                v1 = kv_ps.tile([128, 512], F32, tag="kv", name="v1")
                for d in range(DT):
                    nc.tensor.matmul(
                        v1[:], xk_sb[d][:, tsl], wv_sb[d][:, 0:512],
                        start=(d == 0), stop=(d == DT - 1))
                cp(vp3[:, tt, 0:512], v1[:])
                v2 = kv_ps.tile([128, 512], F32, tag="kv", name="v2")
                for d in range(DT):
                    nc.tensor.matmul(
                        v2[:, 0:256], xk_sb[d][:, tsl], wv_sb[d][:, 512:768],
                        start=(d == 0), stop=(d == DT - 1))
                cp(vp3[:, tt, 512:768], v2[:, 0:256])

        bstack = ExitStack()
        s_ps = bstack.enter_context(
            tc.tile_pool(name="sps", bufs=2, space="PSUM"))
        o_ps = bstack.enter_context(
            tc.tile_pool(name="ops", bufs=1, space="PSUM"))

        # one head's S/exp/AV chain over kt in [kb, ke); (gb, ge) bound the
        # PSUM accumulation group (may span multiple calls on one o_tile)
        def emit_head_seg(h, kb, ke, o_tile, gb=None, ge=None):
            gb = kb if gb is None else gb
            ge = ke if ge is None else ge
            hi, hp = h // 2, (h % 2) * 64
            for kt in range(kb, ke):
                sp = s_ps.tile([128, QS], F32, tag="sp", name="sp")
                for c in range(QC):
                    nc.tensor.matmul(
                        sp[:, c * 512:(c + 1) * 512],
                        kT6[hi][hp:hp + 64, kt * 128:(kt + 1) * 128],
                        qT6[hi][hp:hp + 64, c * 512:(c + 1) * 512],
                        start=True, stop=True, skip_group_check=True)
                p = p_pool.tile([128, QS], MMDT, tag="p", name="p")
                nc.scalar.activation(p[:], sp[:], AF.Exp, scale=0.125)
                first, last = (kt == gb), (kt == ge - 1)
                for qt in range(QT8):
                    q0 = qt * 128
                    # start=True marks the whole 2KB PSUM zero-region
                    # pending-zero, so only the first group per bank starts;
                    # the other groups' first writes overwrite-init via the
                    # pending flag (PE executes in program order).
                    nc.tensor.matmul(
                        o_tile[:, q0:q0 + HD],
                        p[:, q0:q0 + 128],
                        vp3[:, kt, h * HD:(h + 1) * HD],
                        start=first and qt % 4 == 0, stop=last,
                        skip_group_check=True)
                    nc.tensor.matmul(
                        o_tile[:, q0 + HD:q0 + HD + 1],
                        p[:, q0:q0 + 128],
                        mcol_sb[:, kt:kt + 1],
                        start=False, stop=last, skip_group_check=True)

        def o_view(o_tile):
            return o_tile[:].rearrange(
                "p (q s) -> p q s", s=128)[:, :, 0:HD + 1]

        # normalize + transpose head h from its SBUF landing zone
        def emit_head_out(h):
            src = spl[h][:].rearrange("p (q s) -> p q s", s=HD + 1)
            hi, hp = h // 2, (h % 2) * 64
            for qt in range(QT8):
                rcp = nrm_pool.tile([128, 1], F32, tag="rcp", name="rcp")
                nc.vector.reciprocal(rcp[:], src[:, qt, HD:HD + 1])
                osb = nrm_pool.tile([128, HD], MMDT, tag="osb", name="osb")
                nc.vector.tensor_scalar_mul(osb[:], src[:, qt, 0:HD], rcp[:])
                tp = kv_ps.tile([128, 128], MMDT, tag="kv", name="tp")
                nc.tensor.transpose(tp[hp:hp + 64, :], osb[:], id_sb[:])
                nc.vector.tensor_copy(
                    otT6[hi][hp:hp + 64, qt * 128:(qt + 1) * 128],
                    tp[hp:hp + 64, :])

        # ---------------- phase B with KV/Q production interleaved --------
        # finish = (1 DVE op) copy/add O-psum into spl[h], freeing the single
        # O slot fast; normalize+transpose run one head behind, off the
        # critical path.
        def finish(h, si):
            sv = spl[h][:].rearrange("p (q s) -> p q s", s=HD + 1)
            if si == 0:
                nc.vector.tensor_copy(sv, o_view(o_live[h]))
            else:
                nc.vector.tensor_add(sv, sv, o_view(o_live[h]))
            del o_live[h]

        def emit_kv_span(k0, k1):
            k = k0
            while k < k1:
                ke_ = min(k + 512, k1)
                emit_k(k, ke_, range(DT))
                emit_v(k, ke_)
                k = ke_

        # ---------------- segmented self-pacing pipeline -------------------
        # Foreground: the S -> exp -> AV chain per head (exp on ACT is the
        # global bottleneck; it must never starve).  Background (priority
        # pushed far down): all K/V/Q production and the per-head
        # normalize/transpose; the scheduler pops background work whenever
        # an engine would idle.  Segments bound each head's O accumulation
        # span so the single PSUM O slot recycles long before the full K/V
        # production finishes (partials accumulate in SBUF via finish()).
        BG = -1_000_000
        emit_q_pair(0)
        emit_k(0, 128, [0])       # exactly what S(h0, kt0) needs
        emit_v(0, 128)
        with tc.high_priority(offset=BG):
            for p in range(1, DT):
                emit_q_pair(p)
            emit_k(0, 128, range(1, DT))
            emit_kv_span(128, segs[0][1] * 128)

        o_live = {}
        prev = None
        for si, (kb, ke) in enumerate(segs):
            last_seg = si == len(segs) - 1
            for h in range(H):
                if prev is not None:
                    finish(*prev)
                oh = o_ps.tile([128, QS], F32, tag="o", name="o")
                o_live[h] = oh
                emit_head_seg(h, kb, ke, oh)
                if last_seg and h >= 2:
                    with tc.high_priority(offset=BG):
                        emit_head_out(h - 2)
                prev = (h, si)
            if not last_seg:
                with tc.high_priority(offset=BG):
                    emit_kv_span(ke * 128, segs[si + 1][1] * 128)
        finish(*prev)
        emit_head_out(H - 2)
        emit_head_out(H - 1)
        bstack.close()

        # ---------------- phase C: out^T = Wp^T @ O^T + b ------------------
        from contextlib import ExitStack as ES
        with ES() as s:
            cps = s.enter_context(
                tc.tile_pool(name="cps", bufs=3, space="PSUM"))
            for m in range(DT):
                for c in range(QC):
                    ps = cps.tile([128, 512], F32, tag="cps")
                    for p in range(DT):
                        nc.tensor.matmul(
                            ps[:], wp_sb[p][:, m * 128:(m + 1) * 128],
                            otT6[p][:, c * 512:(c + 1) * 512],
                            start=(p == 0), stop=(p == DT - 1))
                    ost = ost_pool.tile([128, 512], MMDT, tag="ost")
                    nc.vector.tensor_scalar_add(ost[:], ps[:],
                                                bp_sb[:, m:m + 1])
                    nc.sync.dma_start(
                        outT[m * 128:(m + 1) * 128, c * 512:(c + 1) * 512],
                        ost[:])


# ---------------------------------------------------------------- host side

@functools.lru_cache(maxsize=None)
def _get_runner(reps: int = 1, nkt: int = DEFAULT_NKT, nseg: int = 1):
    import jax
    from jax.sharding import Mesh, PartitionSpec
    from jax.experimental.shard_map import shard_map

    nc = build_program(reps, nkt, nseg)
    install_neuronx_cc_hook()
    partition_name = (nc.partition_id_tensor.name
                      if nc.partition_id_tensor else None)
    in_names, out_names, out_avals, out_shapes = [], [], [], []
    for alloc in nc.m.functions[0].allocations:
        if not isinstance(alloc, mybir.MemoryLocationSet):
            continue
        name = alloc.memorylocations[0].name
        if alloc.kind == "ExternalInput":
            if name != partition_name:
                in_names.append(name)
        elif alloc.kind == "ExternalOutput":
            out_names.append(name)
            shape = tuple(alloc.tensor_shape)
            dtype = mybir.dt.np(alloc.dtype)
            out_avals.append(jax.core.ShapedArray(shape, dtype))
            out_shapes.append((shape, dtype))
    n_params = len(in_names)
    n_outs = len(out_avals)
    all_in_names = list(in_names) + list(out_names)
    if partition_name is not None:
        all_in_names.append(partition_name)
    donate = tuple(range(n_params, n_params + n_outs))

    def _bodyf(*args):
        operands = list(args)
        if partition_name is not None:
            operands.append(partition_id_tensor())
        outs = _bass_exec_p.bind(
            *operands,
            out_avals=tuple(out_avals),
            in_names=tuple(all_in_names),
            out_names=tuple(out_names),
            lowering_input_output_aliases=(),
            sim_require_finite=True,
            sim_require_nnan=True,
            nc=nc,
        )
        return tuple(outs)

    devices = jax.devices()[:N_CORES]
    mesh = Mesh(np.asarray(devices), ("core",))
    in_specs = (PartitionSpec("core"),) * (n_params + n_outs)
    out_specs = (PartitionSpec("core"),) * len(out_names)
    sharded = jax.jit(
        shard_map(_bodyf, mesh=mesh, in_specs=in_specs, out_specs=out_specs,
                  check_rep=False),
        donate_argnums=donate, keep_unused=True,
    )

    def run(in_maps):
        import jax as _jax
        per_core = [[np.asarray(m[n]) for n in in_names] for m in in_maps]
        concat_in = [np.concatenate([per_core[c][i] for c in range(N_CORES)],
                                    axis=0) for i in range(n_params)]
        concat_zeros = [np.zeros((N_CORES * s[0], *s[1:]), dt)
                        for (s, dt) in out_shapes]
        out_arrs = sharded(*concat_in, *concat_zeros)
        _jax.block_until_ready(out_arrs)
        return [
            {name: np.asarray(out_arrs[i]).reshape(
                N_CORES, *out_shapes[i][0])[c]
             for i, name in enumerate(out_names)}
            for c in range(N_CORES)
        ]

    return run


def make_in_maps(x, mask, w_qkv, w_proj, b_proj):
    import ml_dtypes
    mm_np = ml_dtypes.bfloat16
    x = np.asarray(x, np.float32)
    mask = np.asarray(mask)
    w_qkv = np.asarray(w_qkv, np.float32)
    w_proj = np.asarray(w_proj, np.float32)
    b_proj = np.asarray(b_proj, np.float32)

    keep = [np.nonzero(~mask[b])[0] for b in range(B)]
    nkt = max(DEFAULT_NKT, max(
        (len(k) + 127) // 128 for k in keep))
    nk = nkt * 128

    wqT = np.ascontiguousarray(w_qkv[0:D].T).astype(mm_np)
    wkT = np.ascontiguousarray(w_qkv[D:2 * D].T).astype(mm_np)
    wvT = np.ascontiguousarray(w_qkv[2 * D:3 * D].T).astype(mm_np)
    wpT = np.ascontiguousarray(w_proj.T).astype(mm_np)
    bp = np.ascontiguousarray(b_proj.reshape(DT, 128).T)
    ident = np.eye(128, dtype=mm_np)

    xTs, xTks, mcols = [], [], []
    for b in range(B):
        xT = np.ascontiguousarray(x[b].T).astype(mm_np)
        xTs.append(xT)
        xTk = np.zeros((D, nk), mm_np)
        xTk[:, :len(keep[b])] = xT[:, keep[b]]
        xTks.append(xTk)
        mc = np.zeros((128, nkt), np.float32)
        r = np.arange(128)[:, None] + 128 * np.arange(nkt)[None, :]
        mc[r < len(keep[b])] = 1.0
        mcols.append(mc.astype(mm_np))

    in_maps = []
    for c in range(N_CORES):
        b, qi = divmod(c, CORES_PER_B)
        q0 = qi * QS
        in_maps.append({
            "xTq": np.ascontiguousarray(xTs[b][:, q0:q0 + QS]),
            "xTk": xTks[b],
            "wqT": wqT, "wkT": wkT, "wvT": wvT, "wpT": wpT,
            "bp": bp, "mcol": mcols[b], "ident": ident,
        })
    return in_maps, nkt


def assemble_output(results):
    out = np.empty((B, T, D), np.float32)
    for c in range(N_CORES):
        b, qi = divmod(c, CORES_PER_B)
        q0 = qi * QS
        out[b, q0:q0 + QS, :] = results[c]["outT"].T.astype(np.float32)
    return out


N_SEG = (5, 6, 6)


def kernel(x, mask, w_qkv, w_proj, b_proj):
    in_maps, nkt = make_in_maps(x, mask, w_qkv, w_proj, b_proj)
    run = _get_runner(1, nkt, N_SEG)
    results = run(in_maps)
    return assemble_output(results)



# revision 32
# speedup vs baseline: 1.1522x; 1.1522x over previous
"""Multi-head self-attention Bass/Tile kernel for Trainium2, SPMD over 8 cores.

Problem: B=2, T=4096, D=768, H=12, HD=64 dense MHSA (full TxT scores,
key-padding mask, softmax, out-proj with bias).

Sharding: core c handles batch b=c//4 and query slice q0=(c%4)*1024 for all
12 heads.  No collectives: each core computes a disjoint [768, 1024] slice of
the (transposed) output; the host gathers.

Key optimizations vs the naive layout:
  * Host-side key compaction: ~half the keys are masked out (-inf) in the
    reference; we gather the unmasked key columns of x^T on the host and the
    device kernel only sees NK ~ 2176 keys instead of 4096.  Pad keys have
    zero x columns (so zero K and V rows) and a 0/1 "mcol" vector excludes
    them from the softmax denominator, so no exp bias / mask handling is
    needed on device at all.
  * S = K_h^T.T @ Q_h^T per (head, key-tile) -> PSUM [128k, 1024q], exp on
    ACT -> P (bf16, SBUF).
  * AV is "swapped": O[q, f] accumulates via lhsT=P-chunk [128k, 128q],
    rhs=[V_h | mcol] streaming only 65 columns per (head, key-tile, q-tile)
    instead of streaming 512 query columns.  The denominator comes out as
    column 64 of O for free.
  * Normalize = per-partition scalar multiply on DVE (O is query-major), then
    a PE transpose brings O back to feature-major for the out-projection.
  * Q/K^T/V projections and the out-projection run with full 128-row output
    tiles (head pairs packed), everything SBUF-resident (no DRAM staging).
"""

import functools
import math

import numpy as np

import concourse.bass as bass
import concourse.mybir as mybir
import concourse.tile as tile
from concourse import bacc
from concourse.bass2jax import (
    _bass_exec_p,
    install_neuronx_cc_hook,
    partition_id_tensor,
)

F32 = mybir.dt.float32
BF16 = mybir.dt.bfloat16
MMDT = BF16
AF = mybir.ActivationFunctionType

B, T, D = 2, 4096, 768
H, HD = 12, 64
N_CORES = 8
CORES_PER_B = 4
QS = T // CORES_PER_B           # 1024 query tokens per core
DT = D // 128                   # 6 feature tiles
QC = QS // 512                  # 2 query chunks of 512
QT8 = QS // 128                 # 8 query tiles of 128
DEFAULT_NKT = 17                # key tiles after compaction (seed-0 inputs)

# kt ranges processed per segment of phase B; KV tiles for segment i+1 are
# produced while segment i's heads run (ACT-bound), PSUM only ever holds one
# head's O accumulator.
def _segments(nkt: int, nseg):
    if isinstance(nseg, tuple):          # explicit sizes
        segs, k = [], 0
        for s in nseg:
            if k >= nkt:
                break
            segs.append((k, min(k + s, nkt)))
            k += s
        if k < nkt:
            segs.append((k, nkt))
        return segs
    per = (nkt + nseg - 1) // nseg
    segs = []
    k = 0
    while k < nkt:
        segs.append((k, min(k + per, nkt)))
        k += per
    return segs


def build_program(reps: int = 1, nkt: int = DEFAULT_NKT,
                  nseg: int | None = None):
    nseg = N_SEG if nseg is None else nseg
    nc = bacc.Bacc("TRN2", target_bir_lowering=False, debug=False,
                   num_devices=N_CORES)
    nk = nkt * 128

    xTq = nc.dram_tensor("xTq", [D, QS], MMDT, kind="ExternalInput").ap()
    xTk = nc.dram_tensor("xTk", [D, nk], MMDT, kind="ExternalInput").ap()
    wqT = nc.dram_tensor("wqT", [D, D], MMDT, kind="ExternalInput").ap()
    wkT = nc.dram_tensor("wkT", [D, D], MMDT, kind="ExternalInput").ap()
    wvT = nc.dram_tensor("wvT", [D, D], MMDT, kind="ExternalInput").ap()
    wpT = nc.dram_tensor("wpT", [D, D], MMDT, kind="ExternalInput").ap()
    bp = nc.dram_tensor("bp", [128, DT], F32, kind="ExternalInput").ap()
    mcol = nc.dram_tensor("mcol", [128, nkt], MMDT, kind="ExternalInput").ap()
    ident = nc.dram_tensor("ident", [128, 128], MMDT,
                           kind="ExternalInput").ap()
    outT = nc.dram_tensor("outT", [D, QS], MMDT, kind="ExternalOutput").ap()

    with tile.TileContext(nc) as tc, nc.allow_low_precision(
            reason="bf16 matmul pipeline"):
        def emit_once():
            _body(nc, tc, nkt, nseg, xTq, xTk, wqT, wkT, wvT, wpT, bp, mcol,
                  ident, outT)
        if reps == 1:
            emit_once()
        elif reps < 0:
            for _ in range(-reps):
                emit_once()
        else:
            with tc.For_i(0, reps, 1):
                emit_once()
    nc.compile()
    return nc


def _body(nc, tc, nkt, nseg, xTq, xTk, wqT, wkT, wvT, wpT, bp, mcol, ident,
          outT):
    from contextlib import ExitStack

    nk = nkt * 128
    segs = _segments(nkt, nseg)

    with ExitStack() as root:
        # ---------------- SBUF pools (all root-scoped; it fits) -----------
        const = root.enter_context(tc.tile_pool(name="const", bufs=1))
        w_pool = root.enter_context(tc.tile_pool(name="w", bufs=1))
        x_pool = root.enter_context(tc.tile_pool(name="x", bufs=1))
        qkv_pool = root.enter_context(tc.tile_pool(name="qkv", bufs=1))
        p_pool = root.enter_context(tc.tile_pool(name="p", bufs=15))
        nrm_pool = root.enter_context(tc.tile_pool(name="nrm", bufs=6))
        ot_pool = root.enter_context(tc.tile_pool(name="ot", bufs=1))
        spill_pool = root.enter_context(tc.tile_pool(name="spl", bufs=1))
        ost_pool = root.enter_context(tc.tile_pool(name="ost", bufs=3))

        # DMA transfers all serialize through one modeled DMA pipe
        # (~360GB/s), and each issuing engine pays per-DMA setup on its own
        # sequencer/engine.  So: split out exactly the slices the first
        # S(h0, kt0) chain needs, issue them first across SP/ACT/DVE, and
        # push the bulk through Pool afterwards in need-order.
        wq_sb, wk_sb, wv_sb, wp_sb = [], [], [], []
        for lst, nm in ((wq_sb, "wq"), (wk_sb, "wk"), (wv_sb, "wv"),
                        (wp_sb, "wp")):
            for d in range(DT):
                w = w_pool.tile([128, D], MMDT, tag=f"{nm}{d}", name=f"{nm}{d}")
                lst.append(w)
        xq_sb = [x_pool.tile([128, QS], MMDT, tag=f"xq{d}", name=f"xq{d}")
                 for d in range(DT)]
        xk_sb = [x_pool.tile([128, nk], MMDT, tag=f"xk{d}", name=f"xk{d}")
                 for d in range(DT)]
        bp_sb = const.tile([128, DT], F32, tag="bp")
        mcol_sb = const.tile([128, nkt], MMDT, tag="mcol")
        id_sb = const.tile([128, 128], MMDT, tag="ident")

        # per-head O landing zone in SBUF: segments accumulate into it; the
        # final normalize reads it, so the PSUM O slot frees after one copy
        spl = [spill_pool.tile([128, QT8 * (HD + 1)], F32, tag=f"spl{h}",
                               name=f"spl{h}")
               for h in range(H)]

        for d in range(DT):   # critical: Q pair-0 c0 operands
            nc.sync.dma_start(xq_sb[d][:, 0:512],
                              xTq[d * 128:(d + 1) * 128, 0:512])
            nc.sync.dma_start(wq_sb[d][:, 0:128],
                              wqT[d * 128:(d + 1) * 128, 0:128])
        for d in range(DT):   # critical: K-tile-0 operands + mcol
            nc.scalar.dma_start(wk_sb[d][:, 0:128],
                                wkT[d * 128:(d + 1) * 128, 0:128])
            nc.scalar.dma_start(xk_sb[d][:, 0:512],
                                xTk[d * 128:(d + 1) * 128, 0:512])
        nc.scalar.dma_start(mcol_sb[:], mcol[:])
        for d in range(DT):   # near-critical: rest of Q inputs
            nc.sync.dma_start(xq_sb[d][:, 512:QS],
                              xTq[d * 128:(d + 1) * 128, 512:QS])
        dma = nc.gpsimd.dma_start
        for d in range(DT):   # bulk: K inputs first (gate chunk production)
            dma(xk_sb[d][:, 512:nk], xTk[d * 128:(d + 1) * 128, 512:nk])
            dma(wk_sb[d][:, 128:D], wkT[d * 128:(d + 1) * 128, 128:D])
        for d in range(DT):
            dma(wq_sb[d][:, 128:D], wqT[d * 128:(d + 1) * 128, 128:D])
        for d in range(DT):
            dma(wv_sb[d][:], wvT[d * 128:(d + 1) * 128, :])
        dma(id_sb[:], ident[:])
        dma(bp_sb[:], bp[:])
        for d in range(DT):
            dma(wp_sb[d][:], wpT[d * 128:(d + 1) * 128, :])

        # long-lived activations
        qT6 = [qkv_pool.tile([128, QS], MMDT, tag=f"qT{p}", name=f"qT{p}")
               for p in range(DT)]
        kT6 = [qkv_pool.tile([128, nk], MMDT, tag=f"kT{p}", name=f"kT{p}")
               for p in range(DT)]
        vp_sb = qkv_pool.tile([128, nkt * D], MMDT, tag="vp", name="vp")
        vp3 = vp_sb[:].rearrange("p (kt e) -> p kt e", e=D)
        otT6 = [ot_pool.tile([128, QS], MMDT, tag=f"otT{p}", name=f"otT{p}")
                for p in range(DT)]

        # ---------------- PSUM pools --------------------------------------
        # qps scope closes before the S/O pools open: 2 + (2+4+2) <= 8 banks.
        kv_ps = root.enter_context(
            tc.tile_pool(name="kvps", bufs=2, space="PSUM"))

        def emit_q_pair(p, cp=None):
            cp = cp or nc.vector.tensor_copy
            for c in range(QC):
                ps = kv_ps.tile([128, 512], F32, tag="kv", name="qps")
                for d in range(DT):
                    nc.tensor.matmul(
                        ps[:], wq_sb[d][:, p * 128:(p + 1) * 128],
                        xq_sb[d][:, c * 512:(c + 1) * 512],
                        start=(d == 0), stop=(d == DT - 1))
                cp(qT6[p][:, c * 512:(c + 1) * 512], ps[:])

        # K^T rows (e-tiles) for key span [k0, k1), width <= 512
        def emit_k(k0, k1, es):
            w = k1 - k0
            for e in es:
                kps = kv_ps.tile([128, 512], F32, tag="kv", name="kps")
                for d in range(DT):
                    nc.tensor.matmul(
                        kps[:, 0:w], wk_sb[d][:, e * 128:(e + 1) * 128],
                        xk_sb[d][:, k0:k1],
                        start=(d == 0), stop=(d == DT - 1))
                nc.vector.tensor_copy(kT6[e][:, k0:k1], kps[:, 0:w])

        # V rows for key span [k0, k1) (multiples of 128)
        def emit_v(k0, k1, cp=None):
            cp = cp or nc.vector.tensor_copy
            for tt in range(k0 // 128, k1 // 128):
                tsl = slice(tt * 128, (tt + 1) * 128)
                v1 = kv_ps.tile([128, 512], F32, tag="kv", name="v1")
                for d in range(DT):
                    nc.tensor.matmul(
                        v1[:], xk_sb[d][:, tsl], wv_sb[d][:, 0:512],
                        start=(d == 0), stop=(d == DT - 1))
                cp(vp3[:, tt, 0:512], v1[:])
                v2 = kv_ps.tile([128, 512], F32, tag="kv", name="v2")
                for d in range(DT):
                    nc.tensor.matmul(
                        v2[:, 0:256], xk_sb[d][:, tsl], wv_sb[d][:, 512:768],
                        start=(d == 0), stop=(d == DT - 1))
                cp(vp3[:, tt, 512:768], v2[:, 0:256])

        bstack = ExitStack()
        s_ps = bstack.enter_context(
            tc.tile_pool(name="sps", bufs=2, space="PSUM"))
        o_ps = bstack.enter_context(
            tc.tile_pool(name="ops", bufs=1, space="PSUM"))

        # one head's S/exp/AV chain over kt in [kb, ke); (gb, ge) bound the
        # PSUM accumulation group (may span multiple calls on one o_tile)
        def emit_head_seg(h, kb, ke, o_tile, gb=None, ge=None):
            gb = kb if gb is None else gb
            ge = ke if ge is None else ge
            hi, hp = h // 2, (h % 2) * 64
            for kt in range(kb, ke):
                sp = s_ps.tile([128, QS], F32, tag="sp", name="sp")
                for c in range(QC):
                    nc.tensor.matmul(
                        sp[:, c * 512:(c + 1) * 512],
                        kT6[hi][hp:hp + 64, kt * 128:(kt + 1) * 128],
                        qT6[hi][hp:hp + 64, c * 512:(c + 1) * 512],
                        start=True, stop=True, skip_group_check=True)
                p = p_pool.tile([128, QS], MMDT, tag="p", name="p")
                nc.scalar.activation(p[:], sp[:], AF.Exp, scale=0.125)
                first, last = (kt == gb), (kt == ge - 1)
                for qt in range(QT8):
                    q0 = qt * 128
                    # start=True marks the whole 2KB PSUM zero-region
                    # pending-zero, so only the first group per bank starts;
                    # the other groups' first writes overwrite-init via the
                    # pending flag (PE executes in program order).
                    nc.tensor.matmul(
                        o_tile[:, q0:q0 + HD],
                        p[:, q0:q0 + 128],
                        vp3[:, kt, h * HD:(h + 1) * HD],
                        start=first and qt % 4 == 0, stop=last,
                        skip_group_check=True)
                    nc.tensor.matmul(
                        o_tile[:, q0 + HD:q0 + HD + 1],
                        p[:, q0:q0 + 128],
                        mcol_sb[:, kt:kt + 1],
                        start=False, stop=last, skip_group_check=True)

        def o_view(o_tile):
            return o_tile[:].rearrange(
                "p (q s) -> p q s", s=128)[:, :, 0:HD + 1]

        # normalize + transpose head h from its SBUF landing zone
        def emit_head_out(h):
            src = spl[h][:].rearrange("p (q s) -> p q s", s=HD + 1)
            hi, hp = h // 2, (h % 2) * 64
            for qt in range(QT8):
                rcp = nrm_pool.tile([128, 1], F32, tag="rcp", name="rcp")
                nc.vector.reciprocal(rcp[:], src[:, qt, HD:HD + 1])
                osb = nrm_pool.tile([128, HD], MMDT, tag="osb", name="osb")
                nc.vector.tensor_scalar_mul(osb[:], src[:, qt, 0:HD], rcp[:])
                tp = kv_ps.tile([128, 128], MMDT, tag="kv", name="tp")
                nc.tensor.transpose(tp[hp:hp + 64, :], osb[:], id_sb[:])
                nc.vector.tensor_copy(
                    otT6[hi][hp:hp + 64, qt * 128:(qt + 1) * 128],
                    tp[hp:hp + 64, :])

        # ---------------- phase B with KV/Q production interleaved --------
        # finish = (1 DVE op) copy/add O-psum into spl[h], freeing the single
        # O slot fast; normalize+transpose run one head behind, off the
        # critical path.
        def finish(h, si):
            sv = spl[h][:].rearrange("p (q s) -> p q s", s=HD + 1)
            if si == 0:
                nc.vector.tensor_copy(sv, o_view(o_live[h]))
            else:
                nc.vector.tensor_add(sv, sv, o_view(o_live[h]))
            del o_live[h]

        def emit_kv_span(k0, k1):
            k = k0
            while k < k1:
                ke_ = min(k + 512, k1)
                emit_k(k, ke_, range(DT))
                emit_v(k, ke_)
                k = ke_

        # ---------------- segmented self-pacing pipeline -------------------
        # Foreground: the S -> exp -> AV chain per head (exp on ACT is the
        # global bottleneck; it must never starve).  Background (priority
        # pushed far down): all K/V/Q production and the per-head
        # normalize/transpose; the scheduler pops background work whenever
        # an engine would idle.  Segments bound each head's O accumulation
        # span so the single PSUM O slot recycles long before the full K/V
        # production finishes (partials accumulate in SBUF via finish()).
        BG = -1_000_000
        emit_q_pair(0)
        emit_k(0, 128, [0])       # exactly what S(h0, kt0) needs
        emit_v(0, 128)
        with tc.high_priority(offset=BG):
            for p in range(1, DT):
                emit_q_pair(p)
            emit_k(0, 128, range(1, DT))
            emit_kv_span(128, segs[0][1] * 128)

        o_live = {}
        prev = None
        for si, (kb, ke) in enumerate(segs):
            last_seg = si == len(segs) - 1
            for h in range(H):
                if prev is not None:
                    finish(*prev)
                oh = o_ps.tile([128, QS], F32, tag="o", name="o")
                o_live[h] = oh
                emit_head_seg(h, kb, ke, oh)
                if last_seg and h >= 2:
                    with tc.high_priority(offset=BG):
                        emit_head_out(h - 2)
                prev = (h, si)
            if not last_seg:
                with tc.high_priority(offset=BG):
                    emit_kv_span(ke * 128, segs[si + 1][1] * 128)
        finish(*prev)
        emit_head_out(H - 2)
        emit_head_out(H - 1)
        bstack.close()

        # ---------------- phase C: out^T = Wp^T @ O^T + b ------------------
        from contextlib import ExitStack as ES
        with ES() as s:
            cps = s.enter_context(
                tc.tile_pool(name="cps", bufs=3, space="PSUM"))
            for m in range(DT):
                for c in range(QC):
                    ps = cps.tile([128, 512], F32, tag="cps")
                    for p in range(DT):
                        nc.tensor.matmul(
                            ps[:], wp_sb[p][:, m * 128:(m + 1) * 128],
                            otT6[p][:, c * 512:(c + 1) * 512],
                            start=(p == 0), stop=(p == DT - 1))
                    ost = ost_pool.tile([128, 512], MMDT, tag="ost")
                    nc.vector.tensor_scalar_add(ost[:], ps[:],
                                                bp_sb[:, m:m + 1])
                    nc.sync.dma_start(
                        outT[m * 128:(m + 1) * 128, c * 512:(c + 1) * 512],
                        ost[:])


# ---------------------------------------------------------------- host side

@functools.lru_cache(maxsize=None)
def _get_runner(reps: int = 1, nkt: int = DEFAULT_NKT, nseg: int = 1):
    import jax
    from jax.sharding import Mesh, PartitionSpec
    from jax.experimental.shard_map import shard_map

    nc = build_program(reps, nkt, nseg)
    install_neuronx_cc_hook()
    partition_name = (nc.partition_id_tensor.name
                      if nc.partition_id_tensor else None)
    in_names, out_names, out_avals, out_shapes = [], [], [], []
    for alloc in nc.m.functions[0].allocations:
        if not isinstance(alloc, mybir.MemoryLocationSet):
            continue
        name = alloc.memorylocations[0].name
        if alloc.kind == "ExternalInput":
            if name != partition_name:
                in_names.append(name)
        elif alloc.kind == "ExternalOutput":
            out_names.append(name)
            shape = tuple(alloc.tensor_shape)
            dtype = mybir.dt.np(alloc.dtype)
            out_avals.append(jax.core.ShapedArray(shape, dtype))
            out_shapes.append((shape, dtype))
    n_params = len(in_names)
    n_outs = len(out_avals)
    all_in_names = list(in_names) + list(out_names)
    if partition_name is not None:
        all_in_names.append(partition_name)
    donate = tuple(range(n_params, n_params + n_outs))

    def _bodyf(*args):
        operands = list(args)
        if partition_name is not None:
            operands.append(partition_id_tensor())
        outs = _bass_exec_p.bind(
            *operands,
            out_avals=tuple(out_avals),
            in_names=tuple(all_in_names),
            out_names=tuple(out_names),
            lowering_input_output_aliases=(),
            sim_require_finite=True,
            sim_require_nnan=True,
            nc=nc,
        )
        return tuple(outs)

    devices = jax.devices()[:N_CORES]
    mesh = Mesh(np.asarray(devices), ("core",))
    in_specs = (PartitionSpec("core"),) * (n_params + n_outs)
    out_specs = (PartitionSpec("core"),) * len(out_names)
    sharded = jax.jit(
        shard_map(_bodyf, mesh=mesh, in_specs=in_specs, out_specs=out_specs,
                  check_rep=False),
        donate_argnums=donate, keep_unused=True,
    )

    def run(in_maps):
        import jax as _jax
        per_core = [[np.asarray(m[n]) for n in in_names] for m in in_maps]
        concat_in = [np.concatenate([per_core[c][i] for c in range(N_CORES)],
                                    axis=0) for i in range(n_params)]
        concat_zeros = [np.zeros((N_CORES * s[0], *s[1:]), dt)
                        for (s, dt) in out_shapes]
        out_arrs = sharded(*concat_in, *concat_zeros)
        _jax.block_until_ready(out_arrs)
        return [
            {name: np.asarray(out_arrs[i]).reshape(
                N_CORES, *out_shapes[i][0])[c]
             for i, name in enumerate(out_names)}
            for c in range(N_CORES)
        ]

    return run


def make_in_maps(x, mask, w_qkv, w_proj, b_proj):
    import ml_dtypes
    mm_np = ml_dtypes.bfloat16
    x = np.asarray(x, np.float32)
    mask = np.asarray(mask)
    w_qkv = np.asarray(w_qkv, np.float32)
    w_proj = np.asarray(w_proj, np.float32)
    b_proj = np.asarray(b_proj, np.float32)

    keep = [np.nonzero(~mask[b])[0] for b in range(B)]
    nkt = max(DEFAULT_NKT, max(
        (len(k) + 127) // 128 for k in keep))
    nk = nkt * 128

    wqT = np.ascontiguousarray(w_qkv[0:D].T).astype(mm_np)
    wkT = np.ascontiguousarray(w_qkv[D:2 * D].T).astype(mm_np)
    wvT = np.ascontiguousarray(w_qkv[2 * D:3 * D].T).astype(mm_np)
    wpT = np.ascontiguousarray(w_proj.T).astype(mm_np)
    bp = np.ascontiguousarray(b_proj.reshape(DT, 128).T)
    ident = np.eye(128, dtype=mm_np)

    xTs, xTks, mcols = [], [], []
    for b in range(B):
        xT = np.ascontiguousarray(x[b].T).astype(mm_np)
        xTs.append(xT)
        xTk = np.zeros((D, nk), mm_np)
        xTk[:, :len(keep[b])] = xT[:, keep[b]]
        xTks.append(xTk)
        mc = np.zeros((128, nkt), np.float32)
        r = np.arange(128)[:, None] + 128 * np.arange(nkt)[None, :]
        mc[r < len(keep[b])] = 1.0
        mcols.append(mc.astype(mm_np))

    in_maps = []
    for c in range(N_CORES):
        b, qi = divmod(c, CORES_PER_B)
        q0 = qi * QS
        in_maps.append({
            "xTq": np.ascontiguousarray(xTs[b][:, q0:q0 + QS]),
            "xTk": xTks[b],
            "wqT": wqT, "wkT": wkT, "wvT": wvT, "wpT": wpT,
            "bp": bp, "mcol": mcols[b], "ident": ident,
        })
    return in_maps, nkt


def assemble_output(results):
    out = np.empty((B, T, D), np.float32)
    for c in range(N_CORES):
        b, qi = divmod(c, CORES_PER_B)
        q0 = qi * QS
        out[b, q0:q0 + QS, :] = results[c]["outT"].T.astype(np.float32)
    return out


N_SEG = (5, 6, 6)


def kernel(x, mask, w_qkv, w_proj, b_proj):
    in_maps, nkt = make_in_maps(x, mask, w_qkv, w_proj, b_proj)
    run = _get_runner(1, nkt, N_SEG)
    results = run(in_maps)
    return assemble_output(results)



# revision 33
# speedup vs baseline: 1.1738x; 1.0188x over previous
"""Multi-head self-attention Bass/Tile kernel for Trainium2, SPMD over 8 cores.

Problem: B=2, T=4096, D=768, H=12, HD=64 dense MHSA (full TxT scores,
key-padding mask, softmax, out-proj with bias).

Sharding: core c handles batch b=c//4 and query slice q0=(c%4)*1024 for all
12 heads.  No collectives: each core computes a disjoint [768, 1024] slice of
the (transposed) output; the host gathers.

Key optimizations vs the naive layout:
  * Host-side key compaction: ~half the keys are masked out (-inf) in the
    reference; we gather the unmasked key columns of x^T on the host and the
    device kernel only sees NK ~ 2176 keys instead of 4096.  Pad keys have
    zero x columns (so zero K and V rows) and a 0/1 "mcol" vector excludes
    them from the softmax denominator, so no exp bias / mask handling is
    needed on device at all.
  * S = K_h^T.T @ Q_h^T per (head, key-tile) -> PSUM [128k, 1024q], exp on
    ACT -> P (bf16, SBUF).
  * AV is "swapped": O[q, f] accumulates via lhsT=P-chunk [128k, 128q],
    rhs=[V_h | mcol] streaming only 65 columns per (head, key-tile, q-tile)
    instead of streaming 512 query columns.  The denominator comes out as
    column 64 of O for free.
  * Normalize = per-partition scalar multiply on DVE (O is query-major), then
    a PE transpose brings O back to feature-major for the out-projection.
  * Q/K^T/V projections and the out-projection run with full 128-row output
    tiles (head pairs packed), everything SBUF-resident (no DRAM staging).
"""

import functools
import math

import numpy as np

import concourse.bass as bass
import concourse.mybir as mybir
import concourse.tile as tile
from concourse import bacc
from concourse.bass2jax import (
    _bass_exec_p,
    install_neuronx_cc_hook,
    partition_id_tensor,
)

F32 = mybir.dt.float32
BF16 = mybir.dt.bfloat16
MMDT = BF16
AF = mybir.ActivationFunctionType

B, T, D = 2, 4096, 768
H, HD = 12, 64
N_CORES = 8
CORES_PER_B = 4
QS = T // CORES_PER_B           # 1024 query tokens per core
DT = D // 128                   # 6 feature tiles
QC = QS // 512                  # 2 query chunks of 512
QT8 = QS // 128                 # 8 query tiles of 128
DEFAULT_NKT = 17                # key tiles after compaction (seed-0 inputs)

# kt ranges processed per segment of phase B; KV tiles for segment i+1 are
# produced while segment i's heads run (ACT-bound), PSUM only ever holds one
# head's O accumulator.
def _segments(nkt: int, nseg):
    if isinstance(nseg, tuple):          # explicit sizes
        segs, k = [], 0
        for s in nseg:
            if k >= nkt:
                break
            segs.append((k, min(k + s, nkt)))
            k += s
        if k < nkt:
            segs.append((k, nkt))
        return segs
    per = (nkt + nseg - 1) // nseg
    segs = []
    k = 0
    while k < nkt:
        segs.append((k, min(k + per, nkt)))
        k += per
    return segs


def build_program(reps: int = 1, nkt: int = DEFAULT_NKT,
                  nseg: int | None = None):
    nseg = N_SEG if nseg is None else nseg
    nc = bacc.Bacc("TRN2", target_bir_lowering=False, debug=False,
                   num_devices=N_CORES)
    nk = nkt * 128

    xTq = nc.dram_tensor("xTq", [D, QS], MMDT, kind="ExternalInput").ap()
    xTk = nc.dram_tensor("xTk", [D, nk], MMDT, kind="ExternalInput").ap()
    wqT = nc.dram_tensor("wqT", [D, D], MMDT, kind="ExternalInput").ap()
    wkT = nc.dram_tensor("wkT", [D, D], MMDT, kind="ExternalInput").ap()
    wvT = nc.dram_tensor("wvT", [D, D], MMDT, kind="ExternalInput").ap()
    wpT = nc.dram_tensor("wpT", [D, D], MMDT, kind="ExternalInput").ap()
    bp = nc.dram_tensor("bp", [128, DT], F32, kind="ExternalInput").ap()
    mcol = nc.dram_tensor("mcol", [128, nkt], MMDT, kind="ExternalInput").ap()
    ident = nc.dram_tensor("ident", [128, 128], MMDT,
                           kind="ExternalInput").ap()
    outT = nc.dram_tensor("outT", [D, QS], MMDT, kind="ExternalOutput").ap()

    with tile.TileContext(nc) as tc, nc.allow_low_precision(
            reason="bf16 matmul pipeline"):
        def emit_once():
            _body(nc, tc, nkt, nseg, xTq, xTk, wqT, wkT, wvT, wpT, bp, mcol,
                  ident, outT)
        if reps == 1:
            emit_once()
        elif reps < 0:
            for _ in range(-reps):
                emit_once()
        else:
            with tc.For_i(0, reps, 1):
                emit_once()
    nc.compile()
    return nc


def _body(nc, tc, nkt, nseg, xTq, xTk, wqT, wkT, wvT, wpT, bp, mcol, ident,
          outT):
    from contextlib import ExitStack

    nk = nkt * 128
    segs = _segments(nkt, nseg)

    with ExitStack() as root:
        # ---------------- SBUF pools (all root-scoped; it fits) -----------
        const = root.enter_context(tc.tile_pool(name="const", bufs=1))
        w_pool = root.enter_context(tc.tile_pool(name="w", bufs=1))
        x_pool = root.enter_context(tc.tile_pool(name="x", bufs=1))
        qkv_pool = root.enter_context(tc.tile_pool(name="qkv", bufs=1))
        p_pool = root.enter_context(tc.tile_pool(name="p", bufs=15))
        nrm_pool = root.enter_context(tc.tile_pool(name="nrm", bufs=6))
        ot_pool = root.enter_context(tc.tile_pool(name="ot", bufs=1))
        spill_pool = root.enter_context(tc.tile_pool(name="spl", bufs=1))
        ost_pool = root.enter_context(tc.tile_pool(name="ost", bufs=3))

        # DMA transfers all serialize through one modeled DMA pipe
        # (~360GB/s), and each issuing engine pays per-DMA setup on its own
        # sequencer/engine.  So: split out exactly the slices the first
        # S(h0, kt0) chain needs, issue them first across SP/ACT/DVE, and
        # push the bulk through Pool afterwards in need-order.
        wq_sb, wk_sb, wv_sb, wp_sb = [], [], [], []
        for lst, nm in ((wq_sb, "wq"), (wk_sb, "wk"), (wv_sb, "wv"),
                        (wp_sb, "wp")):
            for d in range(DT):
                w = w_pool.tile([128, D], MMDT, tag=f"{nm}{d}", name=f"{nm}{d}")
                lst.append(w)
        xq_sb = [x_pool.tile([128, QS], MMDT, tag=f"xq{d}", name=f"xq{d}")
                 for d in range(DT)]
        xk_sb = [x_pool.tile([128, nk], MMDT, tag=f"xk{d}", name=f"xk{d}")
                 for d in range(DT)]
        bp_sb = const.tile([128, DT], F32, tag="bp")
        mcol_sb = const.tile([128, nkt], MMDT, tag="mcol")
        id_sb = const.tile([128, 128], MMDT, tag="ident")

        # per-head O landing zone in SBUF: segments accumulate into it; the
        # final normalize reads it, so the PSUM O slot frees after one copy
        spl = [spill_pool.tile([128, QT8 * (HD + 1)], F32, tag=f"spl{h}",
                               name=f"spl{h}")
               for h in range(H)]

        for d in range(DT):   # critical: Q pair-0 c0 operands
            nc.sync.dma_start(xq_sb[d][:, 0:512],
                              xTq[d * 128:(d + 1) * 128, 0:512])
            nc.sync.dma_start(wq_sb[d][:, 0:128],
                              wqT[d * 128:(d + 1) * 128, 0:128])
        for d in range(DT):   # critical: K-tile-0 operands + mcol
            nc.scalar.dma_start(wk_sb[d][:, 0:128],
                                wkT[d * 128:(d + 1) * 128, 0:128])
            nc.scalar.dma_start(xk_sb[d][:, 0:512],
                                xTk[d * 128:(d + 1) * 128, 0:512])
        nc.scalar.dma_start(mcol_sb[:], mcol[:])
        for d in range(DT):   # near-critical: rest of Q inputs
            nc.sync.dma_start(xq_sb[d][:, 512:QS],
                              xTq[d * 128:(d + 1) * 128, 512:QS])
        dma = nc.gpsimd.dma_start
        for d in range(DT):   # bulk: K inputs first (gate chunk production)
            dma(xk_sb[d][:, 512:nk], xTk[d * 128:(d + 1) * 128, 512:nk])
            dma(wk_sb[d][:, 128:D], wkT[d * 128:(d + 1) * 128, 128:D])
        for d in range(DT):
            dma(wq_sb[d][:, 128:D], wqT[d * 128:(d + 1) * 128, 128:D])
        for d in range(DT):
            dma(wv_sb[d][:], wvT[d * 128:(d + 1) * 128, :])
        dma(id_sb[:], ident[:])
        dma(bp_sb[:], bp[:])
        for d in range(DT):
            dma(wp_sb[d][:], wpT[d * 128:(d + 1) * 128, :])

        # long-lived activations
        qT6 = [qkv_pool.tile([128, QS], MMDT, tag=f"qT{p}", name=f"qT{p}")
               for p in range(DT)]
        kT6 = [qkv_pool.tile([128, nk], MMDT, tag=f"kT{p}", name=f"kT{p}")
               for p in range(DT)]
        vp_sb = qkv_pool.tile([128, nkt * D], MMDT, tag="vp", name="vp")
        vp3 = vp_sb[:].rearrange("p (kt e) -> p kt e", e=D)
        otT6 = [ot_pool.tile([128, QS], MMDT, tag=f"otT{p}", name=f"otT{p}")
                for p in range(DT)]

        # ---------------- PSUM pools --------------------------------------
        # qps scope closes before the S/O pools open: 2 + (2+4+2) <= 8 banks.
        kv_ps = root.enter_context(
            tc.tile_pool(name="kvps", bufs=2, space="PSUM"))

        def emit_q_pair(p, cp=None):
            cp = cp or nc.vector.tensor_copy
            for c in range(QC):
                ps = kv_ps.tile([128, 512], F32, tag="kv", name="qps")
                for d in range(DT):
                    nc.tensor.matmul(
                        ps[:], wq_sb[d][:, p * 128:(p + 1) * 128],
                        xq_sb[d][:, c * 512:(c + 1) * 512],
                        start=(d == 0), stop=(d == DT - 1))
                cp(qT6[p][:, c * 512:(c + 1) * 512], ps[:])

        # K^T rows (e-tiles) for key span [k0, k1), width <= 512
        def emit_k(k0, k1, es):
            w = k1 - k0
            for e in es:
                kps = kv_ps.tile([128, 512], F32, tag="kv", name="kps")
                for d in range(DT):
                    nc.tensor.matmul(
                        kps[:, 0:w], wk_sb[d][:, e * 128:(e + 1) * 128],
                        xk_sb[d][:, k0:k1],
                        start=(d == 0), stop=(d == DT - 1))
                nc.vector.tensor_copy(kT6[e][:, k0:k1], kps[:, 0:w])

        # V rows for key span [k0, k1) (multiples of 128)
        def emit_v(k0, k1, cp=None):
            cp = cp or nc.vector.tensor_copy
            for tt in range(k0 // 128, k1 // 128):
                tsl = slice(tt * 128, (tt + 1) * 128)
                v1 = kv_ps.tile([128, 512], F32, tag="kv", name="v1")
                for d in range(DT):
                    nc.tensor.matmul(
                        v1[:], xk_sb[d][:, tsl], wv_sb[d][:, 0:512],
                        start=(d == 0), stop=(d == DT - 1))
                cp(vp3[:, tt, 0:512], v1[:])
                v2 = kv_ps.tile([128, 512], F32, tag="kv", name="v2")
                for d in range(DT):
                    nc.tensor.matmul(
                        v2[:, 0:256], xk_sb[d][:, tsl], wv_sb[d][:, 512:768],
                        start=(d == 0), stop=(d == DT - 1))
                cp(vp3[:, tt, 512:768], v2[:, 0:256])

        bstack = ExitStack()
        s_ps = bstack.enter_context(
            tc.tile_pool(name="sps", bufs=2, space="PSUM"))
        o_ps = bstack.enter_context(
            tc.tile_pool(name="ops", bufs=1, space="PSUM"))

        # one head's S/exp/AV chain over kt in [kb, ke); (gb, ge) bound the
        # PSUM accumulation group (may span multiple calls on one o_tile)
        def emit_head_seg(h, kb, ke, o_tile, gb=None, ge=None):
            gb = kb if gb is None else gb
            ge = ke if ge is None else ge
            hi, hp = h // 2, (h % 2) * 64
            for kt in range(kb, ke):
                sp = s_ps.tile([128, QS], F32, tag="sp", name="sp")
                for c in range(QC):
                    nc.tensor.matmul(
                        sp[:, c * 512:(c + 1) * 512],
                        kT6[hi][hp:hp + 64, kt * 128:(kt + 1) * 128],
                        qT6[hi][hp:hp + 64, c * 512:(c + 1) * 512],
                        start=True, stop=True, skip_group_check=True)
                p = p_pool.tile([128, QS], MMDT, tag="p", name="p")
                nc.scalar.activation(p[:], sp[:], AF.Exp, scale=0.125)
                first, last = (kt == gb), (kt == ge - 1)
                for qt in range(QT8):
                    q0 = qt * 128
                    # start=True marks the whole 2KB PSUM zero-region
                    # pending-zero, so only the first group per bank starts;
                    # the other groups' first writes overwrite-init via the
                    # pending flag (PE executes in program order).
                    nc.tensor.matmul(
                        o_tile[:, q0:q0 + HD],
                        p[:, q0:q0 + 128],
                        vp3[:, kt, h * HD:(h + 1) * HD],
                        start=first and qt % 4 == 0, stop=last,
                        skip_group_check=True)
                    nc.tensor.matmul(
                        o_tile[:, q0 + HD:q0 + HD + 1],
                        p[:, q0:q0 + 128],
                        mcol_sb[:, kt:kt + 1],
                        start=False, stop=last, skip_group_check=True)

        def o_view(o_tile):
            return o_tile[:].rearrange(
                "p (q s) -> p q s", s=128)[:, :, 0:HD + 1]

        # normalize + transpose head h from its SBUF landing zone
        def emit_head_out(h):
            src = spl[h][:].rearrange("p (q s) -> p q s", s=HD + 1)
            hi, hp = h // 2, (h % 2) * 64
            for qt in range(QT8):
                rcp = nrm_pool.tile([128, 1], F32, tag="rcp", name="rcp")
                nc.vector.reciprocal(rcp[:], src[:, qt, HD:HD + 1])
                osb = nrm_pool.tile([128, HD], MMDT, tag="osb", name="osb")
                nc.vector.tensor_scalar_mul(osb[:], src[:, qt, 0:HD], rcp[:])
                tp = kv_ps.tile([128, 128], MMDT, tag="kv", name="tp")
                nc.tensor.transpose(tp[hp:hp + 64, :], osb[:], id_sb[:])
                nc.vector.tensor_copy(
                    otT6[hi][hp:hp + 64, qt * 128:(qt + 1) * 128],
                    tp[hp:hp + 64, :])

        # ---------------- phase B with KV/Q production interleaved --------
        # finish = (1 DVE op) copy/add O-psum into spl[h], freeing the single
        # O slot fast; normalize+transpose run one head behind, off the
        # critical path.
        def finish(h, si):
            sv = spl[h][:].rearrange("p (q s) -> p q s", s=HD + 1)
            if si == 0:
                nc.vector.tensor_copy(sv, o_view(o_live[h]))
            else:
                nc.vector.tensor_add(sv, sv, o_view(o_live[h]))
            del o_live[h]

        def emit_kv_span(k0, k1):
            k = k0
            while k < k1:
                ke_ = min(k + 512, k1)
                emit_k(k, ke_, range(DT))
                emit_v(k, ke_)
                k = ke_

        # ---------------- segmented self-pacing pipeline -------------------
        # Foreground: the S -> exp -> AV chain per head (exp on ACT is the
        # global bottleneck; it must never starve).  Background (priority
        # pushed far down): all K/V/Q production and the per-head
        # normalize/transpose; the scheduler pops background work whenever
        # an engine would idle.  Segments bound each head's O accumulation
        # span so the single PSUM O slot recycles long before the full K/V
        # production finishes (partials accumulate in SBUF via finish()).
        BG = -1_000_000
        emit_q_pair(0)
        emit_k(0, 128, [0])       # exactly what S(h0, kt0) needs
        emit_v(0, 128)
        with tc.high_priority(offset=BG):
            for p in range(1, DT):
                emit_q_pair(p)
            emit_k(0, 128, range(1, DT))
            emit_kv_span(128, segs[0][1] * 128)

        o_live = {}
        prev = None
        for si, (kb, ke) in enumerate(segs):
            last_seg = si == len(segs) - 1
            for h in range(H):
                if prev is not None:
                    finish(*prev)
                oh = o_ps.tile([128, QS], F32, tag="o", name="o")
                o_live[h] = oh
                emit_head_seg(h, kb, ke, oh)
                if last_seg and h >= 2:
                    with tc.high_priority(offset=BG):
                        emit_head_out(h - 2)
                prev = (h, si)
            if not last_seg:
                with tc.high_priority(offset=BG):
                    emit_kv_span(ke * 128, segs[si + 1][1] * 128)
        finish(*prev)
        emit_head_out(H - 2)
        emit_head_out(H - 1)
        bstack.close()

        # ---------------- phase C: out^T = Wp^T @ O^T + b ------------------
        from contextlib import ExitStack as ES
        with ES() as s:
            cps = s.enter_context(
                tc.tile_pool(name="cps", bufs=3, space="PSUM"))
            for m in range(DT):
                for c in range(QC):
                    ps = cps.tile([128, 512], F32, tag="cps")
                    for p in range(DT):
                        nc.tensor.matmul(
                            ps[:], wp_sb[p][:, m * 128:(m + 1) * 128],
                            otT6[p][:, c * 512:(c + 1) * 512],
                            start=(p == 0), stop=(p == DT - 1))
                    ost = ost_pool.tile([128, 512], MMDT, tag="ost")
                    nc.vector.tensor_scalar_add(ost[:], ps[:],
                                                bp_sb[:, m:m + 1])
                    nc.sync.dma_start(
                        outT[m * 128:(m + 1) * 128, c * 512:(c + 1) * 512],
                        ost[:])


# ---------------------------------------------------------------- host side

@functools.lru_cache(maxsize=None)
def _get_runner(reps: int = 1, nkt: int = DEFAULT_NKT, nseg: int = 1):
    import jax
    from jax.sharding import Mesh, PartitionSpec
    from jax.experimental.shard_map import shard_map

    nc = build_program(reps, nkt, nseg)
    install_neuronx_cc_hook()
    partition_name = (nc.partition_id_tensor.name
                      if nc.partition_id_tensor else None)
    in_names, out_names, out_avals, out_shapes = [], [], [], []
    for alloc in nc.m.functions[0].allocations:
        if not isinstance(alloc, mybir.MemoryLocationSet):
            continue
        name = alloc.memorylocations[0].name
        if alloc.kind == "ExternalInput":
            if name != partition_name:
                in_names.append(name)
        elif alloc.kind == "ExternalOutput":
            out_names.append(name)
            shape = tuple(alloc.tensor_shape)
            dtype = mybir.dt.np(alloc.dtype)
            out_avals.append(jax.core.ShapedArray(shape, dtype))
            out_shapes.append((shape, dtype))
    n_params = len(in_names)
    n_outs = len(out_avals)
    all_in_names = list(in_names) + list(out_names)
    if partition_name is not None:
        all_in_names.append(partition_name)
    donate = tuple(range(n_params, n_params + n_outs))

    def _bodyf(*args):
        operands = list(args)
        if partition_name is not None:
            operands.append(partition_id_tensor())
        outs = _bass_exec_p.bind(
            *operands,
            out_avals=tuple(out_avals),
            in_names=tuple(all_in_names),
            out_names=tuple(out_names),
            lowering_input_output_aliases=(),
            sim_require_finite=True,
            sim_require_nnan=True,
            nc=nc,
        )
        return tuple(outs)

    devices = jax.devices()[:N_CORES]
    mesh = Mesh(np.asarray(devices), ("core",))
    in_specs = (PartitionSpec("core"),) * (n_params + n_outs)
    out_specs = (PartitionSpec("core"),) * len(out_names)
    sharded = jax.jit(
        shard_map(_bodyf, mesh=mesh, in_specs=in_specs, out_specs=out_specs,
                  check_rep=False),
        donate_argnums=donate, keep_unused=True,
    )

    def run(in_maps):
        import jax as _jax
        per_core = [[np.asarray(m[n]) for n in in_names] for m in in_maps]
        concat_in = [np.concatenate([per_core[c][i] for c in range(N_CORES)],
                                    axis=0) for i in range(n_params)]
        concat_zeros = [np.zeros((N_CORES * s[0], *s[1:]), dt)
                        for (s, dt) in out_shapes]
        out_arrs = sharded(*concat_in, *concat_zeros)
        _jax.block_until_ready(out_arrs)
        return [
            {name: np.asarray(out_arrs[i]).reshape(
                N_CORES, *out_shapes[i][0])[c]
             for i, name in enumerate(out_names)}
            for c in range(N_CORES)
        ]

    return run


def make_in_maps(x, mask, w_qkv, w_proj, b_proj):
    import ml_dtypes
    mm_np = ml_dtypes.bfloat16
    x = np.asarray(x, np.float32)
    mask = np.asarray(mask)
    w_qkv = np.asarray(w_qkv, np.float32)
    w_proj = np.asarray(w_proj, np.float32)
    b_proj = np.asarray(b_proj, np.float32)

    keep = [np.nonzero(~mask[b])[0] for b in range(B)]
    nkt = max(DEFAULT_NKT, max(
        (len(k) + 127) // 128 for k in keep))
    nk = nkt * 128

    wqT = np.ascontiguousarray(w_qkv[0:D].T).astype(mm_np)
    wkT = np.ascontiguousarray(w_qkv[D:2 * D].T).astype(mm_np)
    wvT = np.ascontiguousarray(w_qkv[2 * D:3 * D].T).astype(mm_np)
    wpT = np.ascontiguousarray(w_proj.T).astype(mm_np)
    bp = np.ascontiguousarray(b_proj.reshape(DT, 128).T)
    ident = np.eye(128, dtype=mm_np)

    xTs, xTks, mcols = [], [], []
    for b in range(B):
        xT = np.ascontiguousarray(x[b].T).astype(mm_np)
        xTs.append(xT)
        xTk = np.zeros((D, nk), mm_np)
        xTk[:, :len(keep[b])] = xT[:, keep[b]]
        xTks.append(xTk)
        mc = np.zeros((128, nkt), np.float32)
        r = np.arange(128)[:, None] + 128 * np.arange(nkt)[None, :]
        mc[r < len(keep[b])] = 1.0
        mcols.append(mc.astype(mm_np))

    in_maps = []
    for c in range(N_CORES):
        b, qi = divmod(c, CORES_PER_B)
        q0 = qi * QS
        in_maps.append({
            "xTq": np.ascontiguousarray(xTs[b][:, q0:q0 + QS]),
            "xTk": xTks[b],
            "wqT": wqT, "wkT": wkT, "wvT": wvT, "wpT": wpT,
            "bp": bp, "mcol": mcols[b], "ident": ident,
        })
    return in_maps, nkt


def assemble_output(results):
    out = np.empty((B, T, D), np.float32)
    for c in range(N_CORES):
        b, qi = divmod(c, CORES_PER_B)
        q0 = qi * QS
        out[b, q0:q0 + QS, :] = results[c]["outT"].T.astype(np.float32)
    return out


N_SEG = (4, 4, 4, 5)


def kernel(x, mask, w_qkv, w_proj, b_proj):
    in_maps, nkt = make_in_maps(x, mask, w_qkv, w_proj, b_proj)
    run = _get_runner(1, nkt, N_SEG)
    results = run(in_maps)
    return assemble_output(results)



# revision 34
# speedup vs baseline: 1.1923x; 1.0157x over previous
"""Multi-head self-attention Bass/Tile kernel for Trainium2, SPMD over 8 cores.

Problem: B=2, T=4096, D=768, H=12, HD=64 dense MHSA (full TxT scores,
key-padding mask, softmax, out-proj with bias).

Sharding: core c handles batch b=c//4 and query slice q0=(c%4)*1024 for all
12 heads.  No collectives: each core computes a disjoint [768, 1024] slice of
the (transposed) output; the host gathers.

Key optimizations vs the naive layout:
  * Host-side key compaction: ~half the keys are masked out (-inf) in the
    reference; we gather the unmasked key columns of x^T on the host and the
    device kernel only sees NK ~ 2176 keys instead of 4096.  Pad keys have
    zero x columns (so zero K and V rows) and a 0/1 "mcol" vector excludes
    them from the softmax denominator, so no exp bias / mask handling is
    needed on device at all.
  * S = K_h^T.T @ Q_h^T per (head, key-tile) -> PSUM [128k, 1024q], exp on
    ACT -> P (bf16, SBUF).
  * AV is "swapped": O[q, f] accumulates via lhsT=P-chunk [128k, 128q],
    rhs=[V_h | mcol] streaming only 65 columns per (head, key-tile, q-tile)
    instead of streaming 512 query columns.  The denominator comes out as
    column 64 of O for free.
  * Normalize = per-partition scalar multiply on DVE (O is query-major), then
    a PE transpose brings O back to feature-major for the out-projection.
  * Q/K^T/V projections and the out-projection run with full 128-row output
    tiles (head pairs packed), everything SBUF-resident (no DRAM staging).
"""

import functools
import math

import numpy as np

import concourse.bass as bass
import concourse.mybir as mybir
import concourse.tile as tile
from concourse import bacc
from concourse.bass2jax import (
    _bass_exec_p,
    install_neuronx_cc_hook,
    partition_id_tensor,
)

F32 = mybir.dt.float32
BF16 = mybir.dt.bfloat16
MMDT = BF16
AF = mybir.ActivationFunctionType

B, T, D = 2, 4096, 768
H, HD = 12, 64
N_CORES = 8
CORES_PER_B = 4
QS = T // CORES_PER_B           # 1024 query tokens per core
DT = D // 128                   # 6 feature tiles
QC = QS // 512                  # 2 query chunks of 512
QT8 = QS // 128                 # 8 query tiles of 128
DEFAULT_NKT = 17                # key tiles after compaction (seed-0 inputs)

# kt ranges processed per segment of phase B; KV tiles for segment i+1 are
# produced while segment i's heads run (ACT-bound), PSUM only ever holds one
# head's O accumulator.
def _segments(nkt: int, nseg):
    if isinstance(nseg, tuple):          # explicit sizes
        segs, k = [], 0
        for s in nseg:
            if k >= nkt:
                break
            segs.append((k, min(k + s, nkt)))
            k += s
        if k < nkt:
            segs.append((k, nkt))
        return segs
    per = (nkt + nseg - 1) // nseg
    segs = []
    k = 0
    while k < nkt:
        segs.append((k, min(k + per, nkt)))
        k += per
    return segs


def build_program(reps: int = 1, nkt: int = DEFAULT_NKT,
                  nseg: int | None = None):
    nseg = N_SEG if nseg is None else nseg
    nc = bacc.Bacc("TRN2", target_bir_lowering=False, debug=False,
                   num_devices=N_CORES)
    nk = nkt * 128

    xTq = nc.dram_tensor("xTq", [D, QS], MMDT, kind="ExternalInput").ap()
    xTk = nc.dram_tensor("xTk", [D, nk], MMDT, kind="ExternalInput").ap()
    wqT = nc.dram_tensor("wqT", [D, D], MMDT, kind="ExternalInput").ap()
    wkT = nc.dram_tensor("wkT", [D, D], MMDT, kind="ExternalInput").ap()
    wvT = nc.dram_tensor("wvT", [D, D], MMDT, kind="ExternalInput").ap()
    wpT = nc.dram_tensor("wpT", [D, D], MMDT, kind="ExternalInput").ap()
    bp = nc.dram_tensor("bp", [128, DT], F32, kind="ExternalInput").ap()
    mcol = nc.dram_tensor("mcol", [128, nkt], MMDT, kind="ExternalInput").ap()
    ident = nc.dram_tensor("ident", [128, 128], MMDT,
                           kind="ExternalInput").ap()
    outT = nc.dram_tensor("outT", [D, QS], MMDT, kind="ExternalOutput").ap()

    with tile.TileContext(nc) as tc, nc.allow_low_precision(
            reason="bf16 matmul pipeline"):
        def emit_once():
            _body(nc, tc, nkt, nseg, xTq, xTk, wqT, wkT, wvT, wpT, bp, mcol,
                  ident, outT)
        if reps == 1:
            emit_once()
        elif reps < 0:
            for _ in range(-reps):
                emit_once()
        else:
            with tc.For_i(0, reps, 1):
                emit_once()
    nc.compile()
    return nc


def _body(nc, tc, nkt, nseg, xTq, xTk, wqT, wkT, wvT, wpT, bp, mcol, ident,
          outT):
    from contextlib import ExitStack

    nk = nkt * 128
    segs = _segments(nkt, nseg)

    with ExitStack() as root:
        # ---------------- SBUF pools (all root-scoped; it fits) -----------
        const = root.enter_context(tc.tile_pool(name="const", bufs=1))
        w_pool = root.enter_context(tc.tile_pool(name="w", bufs=1))
        x_pool = root.enter_context(tc.tile_pool(name="x", bufs=1))
        qkv_pool = root.enter_context(tc.tile_pool(name="qkv", bufs=1))
        p_pool = root.enter_context(tc.tile_pool(name="p", bufs=15))
        nrm_pool = root.enter_context(tc.tile_pool(name="nrm", bufs=6))
        ot_pool = root.enter_context(tc.tile_pool(name="ot", bufs=1))
        spill_pool = root.enter_context(tc.tile_pool(name="spl", bufs=1))
        ost_pool = root.enter_context(tc.tile_pool(name="ost", bufs=3))

        # DMA transfers all serialize through one modeled DMA pipe
        # (~360GB/s), and each issuing engine pays per-DMA setup on its own
        # sequencer/engine.  So: split out exactly the slices the first
        # S(h0, kt0) chain needs, issue them first across SP/ACT/DVE, and
        # push the bulk through Pool afterwards in need-order.
        wq_sb, wk_sb, wv_sb, wp_sb = [], [], [], []
        for lst, nm in ((wq_sb, "wq"), (wk_sb, "wk"), (wv_sb, "wv"),
                        (wp_sb, "wp")):
            for d in range(DT):
                w = w_pool.tile([128, D], MMDT, tag=f"{nm}{d}", name=f"{nm}{d}")
                lst.append(w)
        xq_sb = [x_pool.tile([128, QS], MMDT, tag=f"xq{d}", name=f"xq{d}")
                 for d in range(DT)]
        xk_sb = [x_pool.tile([128, nk], MMDT, tag=f"xk{d}", name=f"xk{d}")
                 for d in range(DT)]
        bp_sb = const.tile([128, DT], F32, tag="bp")
        mcol_sb = const.tile([128, nkt], MMDT, tag="mcol")
        id_sb = const.tile([128, 128], MMDT, tag="ident")

        # per-head O landing zone in SBUF: segments accumulate into it; the
        # final normalize reads it, so the PSUM O slot frees after one copy
        spl = [spill_pool.tile([128, QT8 * (HD + 1)], F32, tag=f"spl{h}",
                               name=f"spl{h}")
               for h in range(H)]

        for d in range(DT):   # critical: Q pair-0 c0 operands
            nc.sync.dma_start(xq_sb[d][:, 0:512],
                              xTq[d * 128:(d + 1) * 128, 0:512])
            nc.sync.dma_start(wq_sb[d][:, 0:128],
                              wqT[d * 128:(d + 1) * 128, 0:128])
        for d in range(DT):   # critical: K-tile-0 operands + mcol
            nc.scalar.dma_start(wk_sb[d][:, 0:128],
                                wkT[d * 128:(d + 1) * 128, 0:128])
            nc.scalar.dma_start(xk_sb[d][:, 0:512],
                                xTk[d * 128:(d + 1) * 128, 0:512])
        nc.scalar.dma_start(mcol_sb[:], mcol[:])
        for d in range(DT):   # near-critical: rest of Q inputs
            nc.sync.dma_start(xq_sb[d][:, 512:QS],
                              xTq[d * 128:(d + 1) * 128, 512:QS])
        dma = nc.gpsimd.dma_start
        for d in range(DT):   # production weights first: seg0 (4 kt) only
            dma(wk_sb[d][:, 128:D], wkT[d * 128:(d + 1) * 128, 128:D])
        for d in range(DT):   # needs the critical key chunk, so Q/K pair
            dma(wq_sb[d][:, 128:D], wqT[d * 128:(d + 1) * 128, 128:D])
        for d in range(DT):   # production gates the pipeline, not bulk keys
            dma(wv_sb[d][:], wvT[d * 128:(d + 1) * 128, :])
        for d in range(DT):
            dma(xk_sb[d][:, 512:nk], xTk[d * 128:(d + 1) * 128, 512:nk])
        dma(id_sb[:], ident[:])
        dma(bp_sb[:], bp[:])
        for d in range(DT):
            dma(wp_sb[d][:], wpT[d * 128:(d + 1) * 128, :])

        # long-lived activations
        qT6 = [qkv_pool.tile([128, QS], MMDT, tag=f"qT{p}", name=f"qT{p}")
               for p in range(DT)]
        kT6 = [qkv_pool.tile([128, nk], MMDT, tag=f"kT{p}", name=f"kT{p}")
               for p in range(DT)]
        vp_sb = qkv_pool.tile([128, nkt * D], MMDT, tag="vp", name="vp")
        vp3 = vp_sb[:].rearrange("p (kt e) -> p kt e", e=D)
        otT6 = [ot_pool.tile([128, QS], MMDT, tag=f"otT{p}", name=f"otT{p}")
                for p in range(DT)]

        # ---------------- PSUM pools --------------------------------------
        # qps scope closes before the S/O pools open: 2 + (2+4+2) <= 8 banks.
        kv_ps = root.enter_context(
            tc.tile_pool(name="kvps", bufs=2, space="PSUM"))

        def emit_q_pair(p, cp=None):
            cp = cp or nc.vector.tensor_copy
            for c in range(QC):
                ps = kv_ps.tile([128, 512], F32, tag="kv", name="qps")
                for d in range(DT):
                    nc.tensor.matmul(
                        ps[:], wq_sb[d][:, p * 128:(p + 1) * 128],
                        xq_sb[d][:, c * 512:(c + 1) * 512],
                        start=(d == 0), stop=(d == DT - 1))
                cp(qT6[p][:, c * 512:(c + 1) * 512], ps[:])

        # K^T rows (e-tiles) for key span [k0, k1), width <= 512
        def emit_k(k0, k1, es):
            w = k1 - k0
            for e in es:
                kps = kv_ps.tile([128, 512], F32, tag="kv", name="kps")
                for d in range(DT):
                    nc.tensor.matmul(
                        kps[:, 0:w], wk_sb[d][:, e * 128:(e + 1) * 128],
                        xk_sb[d][:, k0:k1],
                        start=(d == 0), stop=(d == DT - 1))
                nc.vector.tensor_copy(kT6[e][:, k0:k1], kps[:, 0:w])

        # V rows for key span [k0, k1) (multiples of 128)
        def emit_v(k0, k1, cp=None):
            cp = cp or nc.vector.tensor_copy
            for tt in range(k0 // 128, k1 // 128):
                tsl = slice(tt * 128, (tt + 1) * 128)
                v1 = kv_ps.tile([128, 512], F32, tag="kv", name="v1")
                for d in range(DT):
                    nc.tensor.matmul(
                        v1[:], xk_sb[d][:, tsl], wv_sb[d][:, 0:512],
                        start=(d == 0), stop=(d == DT - 1))
                cp(vp3[:, tt, 0:512], v1[:])
                v2 = kv_ps.tile([128, 512], F32, tag="kv", name="v2")
                for d in range(DT):
                    nc.tensor.matmul(
                        v2[:, 0:256], xk_sb[d][:, tsl], wv_sb[d][:, 512:768],
                        start=(d == 0), stop=(d == DT - 1))
                cp(vp3[:, tt, 512:768], v2[:, 0:256])

        bstack = ExitStack()
        s_ps = bstack.enter_context(
            tc.tile_pool(name="sps", bufs=2, space="PSUM"))
        o_ps = bstack.enter_context(
            tc.tile_pool(name="ops", bufs=1, space="PSUM"))

        # one head's S/exp/AV chain over kt in [kb, ke); (gb, ge) bound the
        # PSUM accumulation group (may span multiple calls on one o_tile)
        def emit_head_seg(h, kb, ke, o_tile, gb=None, ge=None):
            gb = kb if gb is None else gb
            ge = ke if ge is None else ge
            hi, hp = h // 2, (h % 2) * 64
            for kt in range(kb, ke):
                sp = s_ps.tile([128, QS], F32, tag="sp", name="sp")
                for c in range(QC):
                    nc.tensor.matmul(
                        sp[:, c * 512:(c + 1) * 512],
                        kT6[hi][hp:hp + 64, kt * 128:(kt + 1) * 128],
                        qT6[hi][hp:hp + 64, c * 512:(c + 1) * 512],
                        start=True, stop=True, skip_group_check=True)
                p = p_pool.tile([128, QS], MMDT, tag="p", name="p")
                nc.scalar.activation(p[:], sp[:], AF.Exp, scale=0.125)
                first, last = (kt == gb), (kt == ge - 1)
                for qt in range(QT8):
                    q0 = qt * 128
                    # start=True marks the whole 2KB PSUM zero-region
                    # pending-zero, so only the first group per bank starts;
                    # the other groups' first writes overwrite-init via the
                    # pending flag (PE executes in program order).
                    nc.tensor.matmul(
                        o_tile[:, q0:q0 + HD],
                        p[:, q0:q0 + 128],
                        vp3[:, kt, h * HD:(h + 1) * HD],
                        start=first and qt % 4 == 0, stop=last,
                        skip_group_check=True)
                    nc.tensor.matmul(
                        o_tile[:, q0 + HD:q0 + HD + 1],
                        p[:, q0:q0 + 128],
                        mcol_sb[:, kt:kt + 1],
                        start=False, stop=last, skip_group_check=True)

        def o_view(o_tile):
            return o_tile[:].rearrange(
                "p (q s) -> p q s", s=128)[:, :, 0:HD + 1]

        # normalize + transpose head h from its SBUF landing zone
        def emit_head_out(h):
            src = spl[h][:].rearrange("p (q s) -> p q s", s=HD + 1)
            hi, hp = h // 2, (h % 2) * 64
            for qt in range(QT8):
                rcp = nrm_pool.tile([128, 1], F32, tag="rcp", name="rcp")
                nc.vector.reciprocal(rcp[:], src[:, qt, HD:HD + 1])
                osb = nrm_pool.tile([128, HD], MMDT, tag="osb", name="osb")
                nc.vector.tensor_scalar_mul(osb[:], src[:, qt, 0:HD], rcp[:])
                tp = kv_ps.tile([128, 128], MMDT, tag="kv", name="tp")
                nc.tensor.transpose(tp[hp:hp + 64, :], osb[:], id_sb[:])
                nc.vector.tensor_copy(
                    otT6[hi][hp:hp + 64, qt * 128:(qt + 1) * 128],
                    tp[hp:hp + 64, :])

        # ---------------- phase B with KV/Q production interleaved --------
        # finish = (1 DVE op) copy/add O-psum into spl[h], freeing the single
        # O slot fast; normalize+transpose run one head behind, off the
        # critical path.
        def finish(h, si):
            sv = spl[h][:].rearrange("p (q s) -> p q s", s=HD + 1)
            if si == 0:
                nc.vector.tensor_copy(sv, o_view(o_live[h]))
            else:
                nc.vector.tensor_add(sv, sv, o_view(o_live[h]))
            del o_live[h]

        def emit_kv_span(k0, k1):
            k = k0
            while k < k1:
                ke_ = min(k + 512, k1)
                emit_k(k, ke_, range(DT))
                emit_v(k, ke_)
                k = ke_

        # ---------------- segmented self-pacing pipeline -------------------
        # Foreground: the S -> exp -> AV chain per head (exp on ACT is the
        # global bottleneck; it must never starve).  Background (priority
        # pushed far down): all K/V/Q production and the per-head
        # normalize/transpose; the scheduler pops background work whenever
        # an engine would idle.  Segments bound each head's O accumulation
        # span so the single PSUM O slot recycles long before the full K/V
        # production finishes (partials accumulate in SBUF via finish()).
        BG = -1_000_000
        emit_q_pair(0)
        emit_k(0, 128, [0])       # exactly what S(h0, kt0) needs
        emit_v(0, 128)
        with tc.high_priority(offset=BG):
            for p in range(1, DT):
                emit_q_pair(p)
            emit_k(0, 128, range(1, DT))
            emit_kv_span(128, segs[0][1] * 128)

        o_live = {}
        prev = None
        for si, (kb, ke) in enumerate(segs):
            last_seg = si == len(segs) - 1
            for h in range(H):
                if prev is not None:
                    finish(*prev)
                oh = o_ps.tile([128, QS], F32, tag="o", name="o")
                o_live[h] = oh
                emit_head_seg(h, kb, ke, oh)
                if last_seg and h >= 2:
                    with tc.high_priority(offset=BG):
                        emit_head_out(h - 2)
                prev = (h, si)
            if not last_seg:
                with tc.high_priority(offset=BG):
                    emit_kv_span(ke * 128, segs[si + 1][1] * 128)
        finish(*prev)
        emit_head_out(H - 2)
        emit_head_out(H - 1)
        bstack.close()

        # ---------------- phase C: out^T = Wp^T @ O^T + b ------------------
        from contextlib import ExitStack as ES
        with ES() as s:
            cps = s.enter_context(
                tc.tile_pool(name="cps", bufs=3, space="PSUM"))
            for m in range(DT):
                for c in range(QC):
                    ps = cps.tile([128, 512], F32, tag="cps")
                    for p in range(DT):
                        nc.tensor.matmul(
                            ps[:], wp_sb[p][:, m * 128:(m + 1) * 128],
                            otT6[p][:, c * 512:(c + 1) * 512],
                            start=(p == 0), stop=(p == DT - 1))
                    ost = ost_pool.tile([128, 512], MMDT, tag="ost")
                    nc.vector.tensor_scalar_add(ost[:], ps[:],
                                                bp_sb[:, m:m + 1])
                    nc.sync.dma_start(
                        outT[m * 128:(m + 1) * 128, c * 512:(c + 1) * 512],
                        ost[:])


# ---------------------------------------------------------------- host side

@functools.lru_cache(maxsize=None)
def _get_runner(reps: int = 1, nkt: int = DEFAULT_NKT, nseg: int = 1):
    import jax
    from jax.sharding import Mesh, PartitionSpec
    from jax.experimental.shard_map import shard_map

    nc = build_program(reps, nkt, nseg)
    install_neuronx_cc_hook()
    partition_name = (nc.partition_id_tensor.name
                      if nc.partition_id_tensor else None)
    in_names, out_names, out_avals, out_shapes = [], [], [], []
    for alloc in nc.m.functions[0].allocations:
        if not isinstance(alloc, mybir.MemoryLocationSet):
            continue
        name = alloc.memorylocations[0].name
        if alloc.kind == "ExternalInput":
            if name != partition_name:
                in_names.append(name)
        elif alloc.kind == "ExternalOutput":
            out_names.append(name)
            shape = tuple(alloc.tensor_shape)
            dtype = mybir.dt.np(alloc.dtype)
            out_avals.append(jax.core.ShapedArray(shape, dtype))
            out_shapes.append((shape, dtype))
    n_params = len(in_names)
    n_outs = len(out_avals)
    all_in_names = list(in_names) + list(out_names)
    if partition_name is not None:
        all_in_names.append(partition_name)
    donate = tuple(range(n_params, n_params + n_outs))

    def _bodyf(*args):
        operands = list(args)
        if partition_name is not None:
            operands.append(partition_id_tensor())
        outs = _bass_exec_p.bind(
            *operands,
            out_avals=tuple(out_avals),
            in_names=tuple(all_in_names),
            out_names=tuple(out_names),
            lowering_input_output_aliases=(),
            sim_require_finite=True,
            sim_require_nnan=True,
            nc=nc,
        )
        return tuple(outs)

    devices = jax.devices()[:N_CORES]
    mesh = Mesh(np.asarray(devices), ("core",))
    in_specs = (PartitionSpec("core"),) * (n_params + n_outs)
    out_specs = (PartitionSpec("core"),) * len(out_names)
    sharded = jax.jit(
        shard_map(_bodyf, mesh=mesh, in_specs=in_specs, out_specs=out_specs,
                  check_rep=False),
        donate_argnums=donate, keep_unused=True,
    )

    def run(in_maps):
        import jax as _jax
        per_core = [[np.asarray(m[n]) for n in in_names] for m in in_maps]
        concat_in = [np.concatenate([per_core[c][i] for c in range(N_CORES)],
                                    axis=0) for i in range(n_params)]
        concat_zeros = [np.zeros((N_CORES * s[0], *s[1:]), dt)
                        for (s, dt) in out_shapes]
        out_arrs = sharded(*concat_in, *concat_zeros)
        _jax.block_until_ready(out_arrs)
        return [
            {name: np.asarray(out_arrs[i]).reshape(
                N_CORES, *out_shapes[i][0])[c]
             for i, name in enumerate(out_names)}
            for c in range(N_CORES)
        ]

    return run


def make_in_maps(x, mask, w_qkv, w_proj, b_proj):
    import ml_dtypes
    mm_np = ml_dtypes.bfloat16
    x = np.asarray(x, np.float32)
    mask = np.asarray(mask)
    w_qkv = np.asarray(w_qkv, np.float32)
    w_proj = np.asarray(w_proj, np.float32)
    b_proj = np.asarray(b_proj, np.float32)

    keep = [np.nonzero(~mask[b])[0] for b in range(B)]
    nkt = max(DEFAULT_NKT, max(
        (len(k) + 127) // 128 for k in keep))
    nk = nkt * 128

    wqT = np.ascontiguousarray(w_qkv[0:D].T).astype(mm_np)
    wkT = np.ascontiguousarray(w_qkv[D:2 * D].T).astype(mm_np)
    wvT = np.ascontiguousarray(w_qkv[2 * D:3 * D].T).astype(mm_np)
    wpT = np.ascontiguousarray(w_proj.T).astype(mm_np)
    bp = np.ascontiguousarray(b_proj.reshape(DT, 128).T)
    ident = np.eye(128, dtype=mm_np)

    xTs, xTks, mcols = [], [], []
    for b in range(B):
        xT = np.ascontiguousarray(x[b].T).astype(mm_np)
        xTs.append(xT)
        xTk = np.zeros((D, nk), mm_np)
        xTk[:, :len(keep[b])] = xT[:, keep[b]]
        xTks.append(xTk)
        mc = np.zeros((128, nkt), np.float32)
        r = np.arange(128)[:, None] + 128 * np.arange(nkt)[None, :]
        mc[r < len(keep[b])] = 1.0
        mcols.append(mc.astype(mm_np))

    in_maps = []
    for c in range(N_CORES):
        b, qi = divmod(c, CORES_PER_B)
        q0 = qi * QS
        in_maps.append({
            "xTq": np.ascontiguousarray(xTs[b][:, q0:q0 + QS]),
            "xTk": xTks[b],
            "wqT": wqT, "wkT": wkT, "wvT": wvT, "wpT": wpT,
            "bp": bp, "mcol": mcols[b], "ident": ident,
        })
    return in_maps, nkt


def assemble_output(results):
    out = np.empty((B, T, D), np.float32)
    for c in range(N_CORES):
        b, qi = divmod(c, CORES_PER_B)
        q0 = qi * QS
        out[b, q0:q0 + QS, :] = results[c]["outT"].T.astype(np.float32)
    return out


N_SEG = (4, 4, 4, 5)


def kernel(x, mask, w_qkv, w_proj, b_proj):
    in_maps, nkt = make_in_maps(x, mask, w_qkv, w_proj, b_proj)
    run = _get_runner(1, nkt, N_SEG)
    results = run(in_maps)
    return assemble_output(results)

